# revision 21
# baseline (speedup 1.0000x reference)
"""Haar DWT decoder (2-level inverse, zero details) as a Trainium2 Bass kernel.

out[b, c, j, k] = z[b].reshape(C, 128, 128)[c, j//4, k//4] * 0.25
i.e. a 4x4 nearest-neighbor upsample scaled by 1/4.

Data-parallel over batch: 128 samples -> 16 per core on 8 NeuronCores.
"""

import numpy as np

import concourse.bass as bass
import concourse.mybir as mybir
import concourse.tile as tile
from concourse.bass_utils import run_bass_kernel_spmd
# The walrus build in this container rejects instructions carrying more than
# one sync-wait command (codegen: "Too many sync wait commands" — observed on
# a Drain with 3 waits and a DMACopy with 2). Tile freely attaches several
# waits to one instruction, so after tracing we split the excess onto NOPs
# inserted just before the instruction on the same engine; sequential
# dispatch on one engine makes that equivalent.
_MAX_WAITS = 1


def _split_excess_waits(nc: bass.Bass) -> None:
    for f in nc.m.functions:
        for bb in f.blocks:
            insns = bb.instructions
            # Iterate over a snapshot; mutate the live list via insert.
            for ins in list(insns):
                si = ins.sync_info
                if si is None or not si.on_wait or len(si.on_wait) <= _MAX_WAITS:
                    continue
                waits = list(si.on_wait)
                keep = waits[-_MAX_WAITS:]
                spill = waits[:-_MAX_WAITS]
                pos = insns.index(ins)
                nops = []
                for i in range(0, len(spill), _MAX_WAITS):
                    nop = nc.engines[ins.engine].nop(nofuse=True).ins
                    # nop() appended itself to the current bb; pull it out.
                    cur = nc.cur_bb.bb.instructions
                    assert cur[-1] is nop
                    cur.pop()
                    nop.sync_info = mybir.SyncInfo(
                        on_wait=spill[i : i + _MAX_WAITS], on_update=[]
                    )
                    nops.append(nop)
                insns[pos:pos] = nops
                ins.sync_info = mybir.SyncInfo(
                    on_wait=keep, on_update=list(si.on_update)
                )

# Problem constants (hardcoded: module config out_shape=(3,512,512), levels=2)
BATCH = 128
C = 3
CAH = 128  # coarse-approximation spatial dims
CAW = 128
S = 4      # 2**levels upsample factor
H = 512
W = 512
N_CORES = 8
B_SHARD = BATCH // N_CORES  # 16

F32 = mybir.dt.float32

# Pipeline-shape flags, bisected on hardware with interleaved A/B benches
# (see comments at the use sites).  The winning combination is
# RAMP_SPLIT_S0 + TAIL_SPLIT=90: across 14 interleaved measurements it ran a
# stable 156.8-159.6 us, vs a bimodal 141/166-186 us for every alternative
# tried (whole-sample-only DMAs, per-sample half/quarter ring splits,
# scalar-ring-first ordering).
RAMP_SPLIT_S0 = True    # split sample 0 into g3-thirds for an earlier first trigger
TAIL_SPLIT = 90         # partition index to split the last sample across rings
SCALAR_FIRST = False    # give sample 0 to the scalar ring instead of sync
SPLIT_HALVES = 0        # >0: split every sample's DMA into this many partition chunks,
                        # alternating rings (2 = halves, 4 = quarters)
HW_LOADS = False        # issue all input loads on the HW rings up front (they drain
                        # during the preamble/ramp while the rings are otherwise idle)
                        # instead of paced SWDGE loads on gpsimd
HW_LOADS_N = 0          # hybrid: only the first N loads ride the HW rings (landing
                        # ~2 us sooner than SWDGE), the rest stay on paced SWDGE


def _build_nc(b_shard: int = B_SHARD) -> bass.Bass:
    nc = bass.Bass("TRN2", target_bir_lowering=False, debug=False)
    z = nc.dram_tensor("z", [b_shard, C * CAH * CAW], F32, kind="ExternalInput").ap()
    out = nc.dram_tensor("out", [b_shard, C, H, W], F32, kind="ExternalOutput").ap()

    # Layout: flatten out[b] to [(c h), w] = [1536, 512] rows; partition p
    # owns the 12 consecutive rows 12p..12p+11.  DRAM side of the output DMA
    # is then ONE contiguous 24 KiB run per partition (the whole sample is a
    # sequential 3 MiB sweep), i.e. 128 descriptors per sample instead of 384
    # 8-KiB ones — less descriptor-fetch traffic on the rings whose SBUF AXI
    # ports are shared with SDMA engines 7/15 (the prior straggler).
    # Partition p correspondingly needs cA rows G=3p..3p+2 (G = c*128 + jc),
    # i.e. one contiguous 1536 B chunk of z[b]: the load is a clean
    # [128 x 1536 B] sequential sweep as well.
    G3 = 3  # cA rows per partition
    RPP = S * G3  # output rows per partition (12)
    with tile.TileContext(nc) as tc:
        with (
            tc.tile_pool(
                name="zin", bufs=b_shard if HW_LOADS else 6 + HW_LOADS_N
            ) as zin_pool,
            tc.tile_pool(name="wide", bufs=6) as w_pool,
        ):
            # Ring assignment: samples alternate between the two HWDGE rings
            # (sync first); sample 0 is pipelined in g3-thirds so both rings
            # start pumping ~4 us earlier, and the final sample is split
            # unevenly across both rings to equalize their drain time (the
            # scalar ring otherwise finishes ~6 us late).
            # NOTE: do NOT front-load all SWDGE loads (tried: floods the
            # descriptor-fetch AXI port shared with SDMA engine 15, which
            # then straggles ~30 us past the rest).  A 3rd output ring via
            # gpsimd SWDGE also hurts (aggregate drops 433 -> ~360 GB/s).
            def widen(zt, w2, gs):
                """Upsample cA rows gs (slice of 0..2) into w2's rows."""
                zv = zt[:].rearrange("p (g kc) -> p g kc", g=G3)[:, gs, :]
                ng = zv.shape[1]
                w2v = w2[:].rearrange(
                    "p (g jr kc kr) -> p g jr kc kr", g=G3, jr=S, kc=CAW, kr=S
                )[:, gs, :, :, :]
                w2f = w2[:].rearrange(
                    "p (g jr k) -> p g jr k", g=G3, jr=S
                )[:, gs, :, :]
                zb = zv.unsqueeze(3).broadcast_to([CAH, ng, CAW, S])
                nc.vector.tensor_scalar_mul(w2v[:, :, 0, :, :], zb, 0.25)
                nc.scalar.copy(w2f[:, :, 1, :], w2f[:, :, 0, :])
                nc.vector.tensor_copy(w2f[:, :, 2, :], w2f[:, :, 0, :])
                nc.scalar.copy(w2f[:, :, 3, :], w2f[:, :, 0, :])

            zts = {}
            n_ring_loads = b_shard if HW_LOADS else HW_LOADS_N
            if n_ring_loads:
                # These loads go on the HW rings BEFORE any output trigger
                # exists: ring FIFO order then drains them inside the
                # preamble/ramp window where the rings are idle anyway, and
                # they land ~2 us sooner than SWDGE ones.  (Queueing ALL 16
                # here delays the first store packet behind ~3 MiB of load
                # descriptors; a small N keeps the ring clear.)
                for b in range(n_ring_loads):
                    zt = zin_pool.tile([CAH, G3 * CAW], F32)
                    leng = nc.sync if b % 2 == 0 else nc.scalar
                    leng.dma_start(
                        out=zt[:], in_=z[b].rearrange("(p r) -> p r", p=CAH)
                    )
                    zts[b] = zt

            for b in range(b_shard):
                if b in zts:
                    zt = zts[b]
                else:
                    # Load via SWDGE (gpsimd): the HWDGE rings execute FIFO
                    # per ring, so mid-stream loads there would queue behind
                    # multi-MiB output DMAs and stall the pipeline.
                    zt = zin_pool.tile([CAH, G3 * CAW], F32)
                    nc.gpsimd.dma_start(
                        out=zt[:], in_=z[b].rearrange("(p r) -> p r", p=CAH)
                    )

                # Materialize the upsampled sample in SBUF, free layout
                # (g3, jr, k) so the free dim is exactly the 12 output rows
                # in DRAM order; one fully-contiguous 3 MiB DMA per sample
                # (24 KiB per descriptor), alternating between the two HWDGE
                # rings.
                w2 = w_pool.tile([CAH, RPP * W], F32, tag="wide")
                ov = out[b].rearrange("c j k -> (c j k)").rearrange(
                    "(p r) -> p r", p=CAH
                )
                if SCALAR_FIRST:
                    eng = nc.scalar if b % 2 == 0 else nc.sync
                else:
                    eng = nc.sync if b % 2 == 0 else nc.scalar
                if b == 0 and RAMP_SPLIT_S0:
                    # Pipeline the first sample in g3-thirds so the first
                    # output trigger fires right after 1/3 of the widen;
                    # alternate thirds across rings so both start early.
                    other = nc.sync if eng is nc.scalar else nc.scalar
                    for g in range(G3):
                        widen(zt, w2, slice(g, g + 1))
                        teng = (eng, other, eng)[g]
                        teng.dma_start(
                            out=ov[:, g * S * W : (g + 1) * S * W],
                            in_=w2[:, g * S * W : (g + 1) * S * W],
                        )
                elif b == b_shard - 1 and TAIL_SPLIT is not None:
                    # Split the last sample across both rings so they drain
                    # at the same time.
                    widen(zt, w2, slice(0, G3))
                    nc.sync.dma_start(out=ov[:TAIL_SPLIT], in_=w2[:TAIL_SPLIT])
                    nc.scalar.dma_start(out=ov[TAIL_SPLIT:], in_=w2[TAIL_SPLIT:])
                elif SPLIT_HALVES:
                    # Both rings carry an equal share of every sample:
                    # byte-balanced by construction, both busy from the first
                    # sample on, and descriptors stay 24 KiB.
                    widen(zt, w2, slice(0, G3))
                    nchunk = SPLIT_HALVES
                    step = CAH // nchunk
                    for ci in range(nchunk):
                        ceng = nc.sync if ci % 2 == 0 else nc.scalar
                        sl = slice(ci * step, (ci + 1) * step)
                        ceng.dma_start(out=ov[sl], in_=w2[sl])
                else:
                    widen(zt, w2, slice(0, G3))
                    eng.dma_start(out=ov, in_=w2[:])

    _split_excess_waits(nc)
    return nc


_NC_CACHE: dict[int, bass.Bass] = {}


def _get_nc(b_shard: int = B_SHARD) -> bass.Bass:
    if b_shard not in _NC_CACHE:
        _NC_CACHE[b_shard] = _build_nc(b_shard)
    return _NC_CACHE[b_shard]


def kernel(z: np.ndarray) -> np.ndarray:
    z = np.ascontiguousarray(z, dtype=np.float32)
    assert z.shape == (BATCH, C * CAH * CAW), z.shape
    nc = _get_nc()
    in_maps = [
        {"z": z[i * B_SHARD : (i + 1) * B_SHARD]} for i in range(N_CORES)
    ]
    res = run_bass_kernel_spmd(nc, in_maps, list(range(N_CORES)))
    return np.concatenate([res.results[i]["out"] for i in range(N_CORES)], axis=0)



# revision 23
# speedup vs baseline: 1.0223x; 1.0223x over previous
"""Haar DWT decoder (2-level inverse, zero details) as a Trainium2 Bass kernel.

out[b, c, j, k] = z[b].reshape(C, 128, 128)[c, j//4, k//4] * 0.25
i.e. a 4x4 nearest-neighbor upsample scaled by 1/4.

Data-parallel over batch: 128 samples -> 16 per core on 8 NeuronCores.
"""

import numpy as np

import concourse.bass as bass
import concourse.mybir as mybir
import concourse.tile as tile
from concourse.bass_utils import run_bass_kernel_spmd
# The walrus build in this container rejects instructions carrying more than
# one sync-wait command (codegen: "Too many sync wait commands" — observed on
# a Drain with 3 waits and a DMACopy with 2). Tile freely attaches several
# waits to one instruction, so after tracing we split the excess onto NOPs
# inserted just before the instruction on the same engine; sequential
# dispatch on one engine makes that equivalent.
_MAX_WAITS = 1


def _split_excess_waits(nc: bass.Bass) -> None:
    for f in nc.m.functions:
        for bb in f.blocks:
            insns = bb.instructions
            # Iterate over a snapshot; mutate the live list via insert.
            for ins in list(insns):
                si = ins.sync_info
                if si is None or not si.on_wait or len(si.on_wait) <= _MAX_WAITS:
                    continue
                waits = list(si.on_wait)
                keep = waits[-_MAX_WAITS:]
                spill = waits[:-_MAX_WAITS]
                pos = insns.index(ins)
                nops = []
                for i in range(0, len(spill), _MAX_WAITS):
                    nop = nc.engines[ins.engine].nop(nofuse=True).ins
                    # nop() appended itself to the current bb; pull it out.
                    cur = nc.cur_bb.bb.instructions
                    assert cur[-1] is nop
                    cur.pop()
                    nop.sync_info = mybir.SyncInfo(
                        on_wait=spill[i : i + _MAX_WAITS], on_update=[]
                    )
                    nops.append(nop)
                insns[pos:pos] = nops
                ins.sync_info = mybir.SyncInfo(
                    on_wait=keep, on_update=list(si.on_update)
                )

# Problem constants (hardcoded: module config out_shape=(3,512,512), levels=2)
BATCH = 128
C = 3
CAH = 128  # coarse-approximation spatial dims
CAW = 128
S = 4      # 2**levels upsample factor
H = 512
W = 512
N_CORES = 8
B_SHARD = BATCH // N_CORES  # 16

F32 = mybir.dt.float32

# Pipeline-shape flags, bisected on hardware with interleaved A/B benches
# (see comments at the use sites).  The winning combination is
# RAMP_SPLIT_S0 + TAIL_SPLIT=90: across 14 interleaved measurements it ran a
# stable 156.8-159.6 us, vs a bimodal 141/166-186 us for every alternative
# tried (whole-sample-only DMAs, per-sample half/quarter ring splits,
# scalar-ring-first ordering).
RAMP_SPLIT_S0 = True    # split sample 0 into g3-thirds for an earlier first trigger
TAIL_SPLIT = 90         # partition index to split the last sample across rings
SCALAR_FIRST = False    # give sample 0 to the scalar ring instead of sync
SPLIT_HALVES = 0        # >0: split every sample's DMA into this many partition chunks,
                        # alternating rings (2 = halves, 4 = quarters)
HW_LOADS = False        # issue all input loads on the HW rings up front (they drain
                        # during the preamble/ramp while the rings are otherwise idle)
                        # instead of paced SWDGE loads on gpsimd
HW_LOADS_N = 0          # hybrid: only the first N loads ride the HW rings (landing
                        # ~2 us sooner than SWDGE), the rest stay on paced SWDGE
WIDE_BUFS = 6           # wide-pool depth (how many 3 MiB samples may be in flight)


def _build_nc(b_shard: int = B_SHARD) -> bass.Bass:
    nc = bass.Bass("TRN2", target_bir_lowering=False, debug=False)
    z = nc.dram_tensor("z", [b_shard, C * CAH * CAW], F32, kind="ExternalInput").ap()
    out = nc.dram_tensor("out", [b_shard, C, H, W], F32, kind="ExternalOutput").ap()

    # Layout: flatten out[b] to [(c h), w] = [1536, 512] rows; partition p
    # owns the 12 consecutive rows 12p..12p+11.  DRAM side of the output DMA
    # is then ONE contiguous 24 KiB run per partition (the whole sample is a
    # sequential 3 MiB sweep), i.e. 128 descriptors per sample instead of 384
    # 8-KiB ones — less descriptor-fetch traffic on the rings whose SBUF AXI
    # ports are shared with SDMA engines 7/15 (the prior straggler).
    # Partition p correspondingly needs cA rows G=3p..3p+2 (G = c*128 + jc),
    # i.e. one contiguous 1536 B chunk of z[b]: the load is a clean
    # [128 x 1536 B] sequential sweep as well.
    G3 = 3  # cA rows per partition
    RPP = S * G3  # output rows per partition (12)
    with tile.TileContext(nc) as tc:
        with (
            tc.tile_pool(
                name="zin", bufs=b_shard if HW_LOADS else 6 + HW_LOADS_N
            ) as zin_pool,
            tc.tile_pool(name="wide", bufs=WIDE_BUFS) as w_pool,
        ):
            # Ring assignment: samples alternate between the two HWDGE rings
            # (sync first); sample 0 is pipelined in g3-thirds so both rings
            # start pumping ~4 us earlier, and the final sample is split
            # unevenly across both rings to equalize their drain time (the
            # scalar ring otherwise finishes ~6 us late).
            # NOTE: do NOT front-load all SWDGE loads (tried: floods the
            # descriptor-fetch AXI port shared with SDMA engine 15, which
            # then straggles ~30 us past the rest).  A 3rd output ring via
            # gpsimd SWDGE also hurts (aggregate drops 433 -> ~360 GB/s).
            def widen(zt, w2, gs):
                """Upsample cA rows gs (slice of 0..2) into w2's rows."""
                zv = zt[:].rearrange("p (g kc) -> p g kc", g=G3)[:, gs, :]
                ng = zv.shape[1]
                w2v = w2[:].rearrange(
                    "p (g jr kc kr) -> p g jr kc kr", g=G3, jr=S, kc=CAW, kr=S
                )[:, gs, :, :, :]
                w2f = w2[:].rearrange(
                    "p (g jr k) -> p g jr k", g=G3, jr=S
                )[:, gs, :, :]
                zb = zv.unsqueeze(3).broadcast_to([CAH, ng, CAW, S])
                nc.vector.tensor_scalar_mul(w2v[:, :, 0, :, :], zb, 0.25)
                nc.scalar.copy(w2f[:, :, 1, :], w2f[:, :, 0, :])
                nc.vector.tensor_copy(w2f[:, :, 2, :], w2f[:, :, 0, :])
                nc.scalar.copy(w2f[:, :, 3, :], w2f[:, :, 0, :])

            zts = {}
            n_ring_loads = b_shard if HW_LOADS else HW_LOADS_N
            if n_ring_loads:
                # These loads go on the HW rings BEFORE any output trigger
                # exists: ring FIFO order then drains them inside the
                # preamble/ramp window where the rings are idle anyway, and
                # they land ~2 us sooner than SWDGE ones.  (Queueing ALL 16
                # here delays the first store packet behind ~3 MiB of load
                # descriptors; a small N keeps the ring clear.)
                for b in range(n_ring_loads):
                    zt = zin_pool.tile([CAH, G3 * CAW], F32)
                    leng = nc.sync if b % 2 == 0 else nc.scalar
                    leng.dma_start(
                        out=zt[:], in_=z[b].rearrange("(p r) -> p r", p=CAH)
                    )
                    zts[b] = zt

            for b in range(b_shard):
                if b in zts:
                    zt = zts[b]
                else:
                    # Load via SWDGE (gpsimd): the HWDGE rings execute FIFO
                    # per ring, so mid-stream loads there would queue behind
                    # multi-MiB output DMAs and stall the pipeline.
                    zt = zin_pool.tile([CAH, G3 * CAW], F32)
                    nc.gpsimd.dma_start(
                        out=zt[:], in_=z[b].rearrange("(p r) -> p r", p=CAH)
                    )

                # Materialize the upsampled sample in SBUF, free layout
                # (g3, jr, k) so the free dim is exactly the 12 output rows
                # in DRAM order; one fully-contiguous 3 MiB DMA per sample
                # (24 KiB per descriptor), alternating between the two HWDGE
                # rings.
                w2 = w_pool.tile([CAH, RPP * W], F32, tag="wide")
                ov = out[b].rearrange("c j k -> (c j k)").rearrange(
                    "(p r) -> p r", p=CAH
                )
                if SCALAR_FIRST:
                    eng = nc.scalar if b % 2 == 0 else nc.sync
                else:
                    eng = nc.sync if b % 2 == 0 else nc.scalar
                if b == 0 and RAMP_SPLIT_S0:
                    # Pipeline the first sample in g3-thirds so the first
                    # output trigger fires right after 1/3 of the widen;
                    # alternate thirds across rings so both start early.
                    other = nc.sync if eng is nc.scalar else nc.scalar
                    for g in range(G3):
                        widen(zt, w2, slice(g, g + 1))
                        teng = (eng, other, eng)[g]
                        teng.dma_start(
                            out=ov[:, g * S * W : (g + 1) * S * W],
                            in_=w2[:, g * S * W : (g + 1) * S * W],
                        )
                elif b == b_shard - 1 and TAIL_SPLIT is not None:
                    # Split the last sample across both rings so they drain
                    # at the same time.
                    widen(zt, w2, slice(0, G3))
                    nc.sync.dma_start(out=ov[:TAIL_SPLIT], in_=w2[:TAIL_SPLIT])
                    nc.scalar.dma_start(out=ov[TAIL_SPLIT:], in_=w2[TAIL_SPLIT:])
                elif SPLIT_HALVES:
                    # Both rings carry an equal share of every sample:
                    # byte-balanced by construction, both busy from the first
                    # sample on, and descriptors stay 24 KiB.
                    widen(zt, w2, slice(0, G3))
                    nchunk = SPLIT_HALVES
                    step = CAH // nchunk
                    for ci in range(nchunk):
                        ceng = nc.sync if ci % 2 == 0 else nc.scalar
                        sl = slice(ci * step, (ci + 1) * step)
                        ceng.dma_start(out=ov[sl], in_=w2[sl])
                else:
                    widen(zt, w2, slice(0, G3))
                    eng.dma_start(out=ov, in_=w2[:])

    _split_excess_waits(nc)
    return nc


_NC_CACHE: dict[int, bass.Bass] = {}


def _get_nc(b_shard: int = B_SHARD) -> bass.Bass:
    if b_shard not in _NC_CACHE:
        _NC_CACHE[b_shard] = _build_nc(b_shard)
    return _NC_CACHE[b_shard]


def kernel(z: np.ndarray) -> np.ndarray:
    z = np.ascontiguousarray(z, dtype=np.float32)
    assert z.shape == (BATCH, C * CAH * CAW), z.shape
    nc = _get_nc()
    in_maps = [
        {"z": z[i * B_SHARD : (i + 1) * B_SHARD]} for i in range(N_CORES)
    ]
    res = run_bass_kernel_spmd(nc, in_maps, list(range(N_CORES)))
    return np.concatenate([res.results[i]["out"] for i in range(N_CORES)], axis=0)

